# revision 25
# baseline (speedup 1.0000x reference)
"""Multi-head self-attention (B=4, S=2048, D=1024, H=16, Hd=64) on 8 TRN2 cores.

Sharding: tensor-parallel over heads. Core c owns heads 2c, 2c+1:
  - computes Q^T/K^T [128hd, tok] (f32r) and V (bf16) for its 2 heads
  - flash-style attention per (batch, head): S^T = K^T.T @ Q^T into
    [128,1024] psum stripes, exp on ACT (scale=1/8 folded in), AV +
    denominator via ones-append (M=65), reciprocal_approx_fast +
    PE ones-broadcast, normalize on DVE -> A^T (bf16)
  - 8 pipelined AllGathers (one per batch x local-head row-block)
  - out-proj: out^T[:, c-slice] = wo_perm_c.T @ A'^T + bo_c (bf16 matmul,
    fp32 psum); host transposes/concats column slices.

Matmul dtype: float32r (~1.5e-4 rel err, full PE rate at N>=512) for
projections/scores; bf16 for the P/V/AV/out-proj path (psum always fp32).
"""
import numpy as np

B, S, D, H, HD = 4, 2048, 1024, 16, 64
N_CORES = 8
TOK = B * S            # 8192
HPC = H // N_CORES     # 2 heads per core
CW = HPC * HD          # 128 cols per core
QS = 1024              # query stripe
NKT = S // 128         # 16 kt chunks per batch
NQS = S // QS          # 2 q stripes per batch
NTB = TOK // 512       # 16 token blocks overall

_CACHE = {}


def _build():
    import concourse.bacc as bacc
    import concourse.mybir as mybir
    import concourse.tile as tile

    F32 = mybir.dt.float32
    F32R = mybir.dt.float32r
    BF16 = mybir.dt.bfloat16
    AF = mybir.ActivationFunctionType

    nc = bacc.Bacc(trn_type="TRN2", target_bir_lowering=False, debug=False,
                   num_devices=N_CORES)

    xT = nc.dram_tensor("xT", [D, TOK], F32, kind="ExternalInput")
    wq = nc.dram_tensor("wq", [D, CW], F32, kind="ExternalInput")
    wk = nc.dram_tensor("wk", [D, CW], F32, kind="ExternalInput")
    wv = nc.dram_tensor("wv", [D, CW], F32, kind="ExternalInput")
    wo = nc.dram_tensor("wo", [D, CW], F32, kind="ExternalInput")  # row-permuted
    bq = nc.dram_tensor("bq", [CW, 1], F32, kind="ExternalInput")
    bk = nc.dram_tensor("bk", [CW, 1], F32, kind="ExternalInput")
    bv = nc.dram_tensor("bv", [CW, 1], F32, kind="ExternalInput")
    bo = nc.dram_tensor("bo", [CW, 1], F32, kind="ExternalInput")
    ident = nc.dram_tensor("ident", [128, 128], F32, kind="ExternalInput")
    outT = nc.dram_tensor("outT", [CW, TOK], F32, kind="ExternalOutput")

    with tile.TileContext(nc) as tc:
        with tc.tile_pool(name="sb", bufs=1) as sb, \
             tc.tile_pool(name="dram", bufs=1, space="DRAM") as dram:
            # ---------------- prologue: weights, biases, constants --------
            w_r = {}
            for wname, wdram, odt in (("wq", wq, F32R), ("wk", wk, F32R),
                                      ("wv", wv, F32R), ("wo", wo, BF16)):
                wr = sb.tile([128, D], odt, tag=f"{wname}_r",
                             name=f"{wname}_r")
                wsrc = wdram.ap().rearrange("(k p) m -> p k m", p=128)
                for half in range(2):
                    wst = sb.tile([128, 512], F32, tag="xstage", bufs=6,
                                  name=f"wst_{wname}{half}")
                    nc.sync.dma_start(
                        wst[:].rearrange("p (k m) -> p k m", k=4),
                        wsrc[:, half * 4:half * 4 + 4, :])
                    nc.vector.tensor_copy(
                        wr[:, half * 512:(half + 1) * 512], wst[:])
                w_r[wname] = wr
            wq_r, wk_r, wv_r, wo_b = w_r["wq"], w_r["wk"], w_r["wv"], w_r["wo"]

            idst = sb.tile([128, 512], F32, tag="xstage", bufs=6, name="idst")
            nc.sync.dma_start(idst[:, 0:128], ident[:])
            identr = sb.tile([128, 128], F32R, tag="identr", name="identr")
            nc.vector.tensor_copy(identr[:], idst[:, 0:128])

            bias_t = {}
            for bname, bdram in (("bq", bq), ("bk", bk), ("bv", bv),
                                 ("bo", bo)):
                bt_ = sb.tile([CW, 1], F32, tag=f"{bname}_t", name=f"{bname}_t")
                nc.sync.dma_start(bt_[:], bdram[:])
                bias_t[bname] = bt_

            ones_f = sb.tile([65, 64], F32, tag="ones_f", name="ones_f")
            nc.vector.memset(ones_f[:], 1.0)
            ones_r = sb.tile([65, 64], F32R, tag="ones_r", name="ones_r")
            nc.vector.tensor_copy(ones_r[:], ones_f[:])

            agin = {}
            agout = {}
            for b in range(B):
                for h in range(2):
                    for q in range(NQS):
                        agin[(b, h, q)] = dram.tile(
                            [64, QS], BF16, tag=f"agi{b}{h}{q}",
                            name=f"agi{b}{h}{q}")
                        agout[(b, h, q)] = dram.tile(
                            [64 * N_CORES, QS], BF16, tag=f"ago{b}{h}{q}",
                            addr_space="Shared", name=f"ago{b}{h}{q}")

            with tc.tile_pool(name="ps12", bufs=1, space="PSUM") as ps:
                qkv = {}
                xr_tiles = {}
                vext = {}
                at_tiles = {}

                def emit_p1_loads(b, tb):
                    if tb == 0:
                        qkv[b] = (
                            sb.tile([128, S], F32R, tag="qt_sb", bufs=2,
                                    name=f"qt{b}"),
                            sb.tile([128, S], F32R, tag="kt_sb", bufs=2,
                                    name=f"kt{b}"),
                            sb.tile([128, S], F32R, tag="vt_sb", bufs=2,
                                    name=f"vt{b}"),
                        )
                    g0 = b * S + tb * 512
                    xr = []
                    for k in range(8):
                        xs = sb.tile([128, 512], F32, tag="xstage", bufs=6,
                                     name=f"xs{b}_{tb}_{k}")
                        nc.sync.dma_start(
                            xs[:], xT[k * 128:(k + 1) * 128, g0:g0 + 512])
                        xk = sb.tile([128, 512], F32R, tag="xr", bufs=18,
                                     name=f"xr{b}_{tb}_{k}")
                        nc.vector.tensor_copy(xk[:], xs[:])
                        xr.append(xk)
                    xr_tiles[(b, tb)] = xr

                def emit_p1_group(b, tb, which):
                    qt, kt, vt = qkv[b]
                    xr = xr_tiles[(b, tb)]
                    w_, out_sb, bias = (
                        (wq_r, qt, bias_t["bq"]),
                        (wk_r, kt, bias_t["bk"]),
                        (wv_r, vt, bias_t["bv"]))[which]
                    pp = ps.tile([128, 512], F32, tag="proj", bufs=2,
                                 name=f"pp{b}_{tb}_{which}")
                    for k in range(8):
                        nc.tensor.matmul(
                            pp[:], w_[:, k * 128:(k + 1) * 128],
                            xr[k][:], start=(k == 0), stop=(k == 7))
                    nc.vector.tensor_scalar_add(
                        out_sb[:, tb * 512:(tb + 1) * 512], pp[:], bias[:])

                def emit_vext_chunk(b, tbi):
                    vt = qkv[b][2]
                    for ktc in range(4 * tbi, 4 * tbi + 4):
                        tp = ps.tile([128, 128], F32R, tag="proj", bufs=2,
                                     name=f"tp{b}_{ktc}")
                        nc.tensor.transpose(
                            tp[:], vt[:, ktc * 128:(ktc + 1) * 128],
                            identr[:])
                        for h in range(2):
                            ve = sb.tile([128, 128], BF16, tag="vext",
                                         bufs=36, name=f"ve{b}_{ktc}_{h}")
                            nc.vector.memset(ve[:, 0:64], 1.0)
                            nc.vector.tensor_copy(
                                ve[:, 64:128],
                                tp[:, h * 64:(h + 1) * 64])
                            vext[(b, ktc, h)] = ve

                def emit_p2_stripe(b, h, qs_i, jobs):
                    qt, kt, vt = qkv[b]
                    if qs_i == 0:
                        at_tiles[(b, h)] = sb.tile(
                            [128, S], BF16, tag="at_t", bufs=2,
                            name=f"at{b}_{h}")
                    at_t = at_tiles[(b, h)]
                    q0 = qs_i * QS
                    pav = ps.tile([128, QS], F32, tag="av", bufs=1,
                                  name=f"pav{b}_{h}_{qs_i}")
                    def emit_av(ktc, pt):
                        for half in range(2):
                            nc.tensor.matmul(
                                pav[:, half * 512:(half + 1) * 512],
                                vext[(b, ktc, h)][:],
                                pt[:, half * 512:(half + 1) * 512],
                                start=(ktc == 0), stop=(ktc == NKT - 1))

                    for ktc in range(NKT):
                        if ktc in jobs:
                            jobs[ktc]()
                        s_ps = ps.tile([128, QS], F32, tag="s", bufs=2,
                                       name=f"s{b}{h}{qs_i}{ktc}")
                        for half in range(2):
                            nc.tensor.matmul(
                                s_ps[:, half * 512:(half + 1) * 512],
                                kt[h * 64:(h + 1) * 64,
                                   ktc * 128:(ktc + 1) * 128],
                                qt[h * 64:(h + 1) * 64,
                                   q0 + half * 512:q0 + (half + 1) * 512],
                                start=True, stop=True,
                                tile_position=(h * 64, 0))
                        pt = sb.tile([128, QS], BF16, tag="p_sb",
                                     bufs=3, name=f"p{b}{h}{qs_i}{ktc}")
                        nc.scalar.activation(pt[:], s_ps[:], AF.Exp,
                                             scale=0.125)
                        emit_av(ktc, pt)
                    araw = sb.tile([128, QS], F32, tag="araw", bufs=3,
                                   name=f"ar{b}_{h}_{qs_i}")
                    nc.vector.tensor_copy(araw[:], pav[:])
                    rcf = sb.tile([128, QS], F32, tag="rcf", bufs=3,
                                  name=f"rcf{b}_{h}_{qs_i}")
                    nc.vector.reciprocal_approx_fast(rcf[:], araw[:])
                    bcs = sb.tile([128, QS], F32, tag="bcs", bufs=2,
                                  name=f"bcs{b}_{h}_{qs_i}")
                    nc.gpsimd.partition_broadcast(bcs[:], rcf[0:1, :])
                    nc.vector.tensor_mul(at_t[64:128, q0:q0 + QS],
                                         araw[64:128, :], bcs[64:128, :])
                    nc.sync.dma_start(agin[(b, h, qs_i)][:],
                                      at_t[64:128, q0:q0 + QS])
                    nc.gpsimd.collective_compute(
                        "AllGather", mybir.AluOpType.bypass,
                        replica_groups=[list(range(N_CORES))],
                        ins=[agin[(b, h, qs_i)][:]],
                        outs=[agout[(b, h, qs_i)][:]],
                    )

                def emit_p3_tb(tb):
                    bb = tb // 4
                    qsb = (tb % 4) // 2
                    hf = tb % 2
                    c0 = hf * 512
                    po = ps.tile([128, 512], F32, tag="proj", bufs=2,
                                 name=f"po{tb}")
                    for kc in range(8):
                        ast = sb.tile([128, 512], BF16, tag="ast", bufs=6,
                                      name=f"ast{tb}_{kc}")
                        src = agout[(bb, kc // 4, qsb)]
                        r0 = (kc % 4) * 128
                        nc.sync.dma_start(ast[:],
                                          src[r0:r0 + 128, c0:c0 + 512])
                        nc.tensor.matmul(po[:],
                                         wo_b[:, kc * 128:(kc + 1) * 128],
                                         ast[:], start=(kc == 0),
                                         stop=(kc == 7))
                    ot = sb.tile([128, 512], F32, tag="ot", bufs=3,
                                 name=f"ot{tb}")
                    nc.vector.tensor_scalar_add(ot[:], po[:], bias_t["bo"][:])
                    nc.sync.dma_start(outT[:, tb * 512:(tb + 1) * 512], ot[:])

                # batch 0 projections up front
                for tb in range(4):
                    emit_p1_loads(0, tb)
                    for w in range(3):
                        emit_p1_group(0, tb, w)
                for tbi in range(4):
                    emit_vext_chunk(0, tbi)

                for b in range(B):
                    for h in range(2):
                        for qs_i in range(NQS):
                            i = h * NQS + qs_i
                            jobs = {}
                            # P3 token-blocks woven into later batches' stripes
                            p3_sched = {(1, 1): [0], (1, 2): [1], (1, 3): [2],
                                        (2, 0): [3], (2, 1): [4], (2, 2): [5],
                                        (2, 3): [6], (3, 0): [7],
                                        (3, 1): [8, 9], (3, 2): [10, 11],
                                        (3, 3): [12, 13]}
                            for j, tb3 in enumerate(p3_sched.get((b, i), [])):
                                jobs[5 + 8 * j] = (lambda tb=tb3:
                                                   emit_p3_tb(tb))
                            if b + 1 < B:
                                jobs[0] = (lambda bb=b + 1, tb=i:
                                           emit_p1_loads(bb, tb))
                                if i >= 1:
                                    jobs[3] = (lambda bb=b + 1, tb=i - 1:
                                               emit_p1_group(bb, tb, 0))
                                    jobs[7] = (lambda bb=b + 1, tb=i - 1:
                                               emit_p1_group(bb, tb, 1))
                                    jobs[11] = (lambda bb=b + 1, tb=i - 1:
                                                emit_p1_group(bb, tb, 2))
                                if i >= 2:
                                    jobs[14] = (lambda bb=b + 1, tbi=i - 2:
                                                emit_vext_chunk(bb, tbi))
                            emit_p2_stripe(b, h, qs_i, jobs)
                    if b + 1 < B:
                        # tail of next batch's projections
                        for w in range(3):
                            emit_p1_group(b + 1, 3, w)
                        emit_vext_chunk(b + 1, 2)
                        emit_vext_chunk(b + 1, 3)

                # ------------- P3 tail: last token blocks -----------------
                for tb in range(14, NTB):
                    emit_p3_tb(tb)

    nc.compile()
    return nc


def _get_nc():
    if "nc" not in _CACHE:
        _CACHE["nc"] = _build()
    return _CACHE["nc"]


def _make_in_maps(x, Wq, bq, Wk, bk, Wv, bv, Wo, bo):
    x = np.asarray(x, dtype=np.float32)
    Wq, Wk, Wv, Wo = (np.asarray(w, dtype=np.float32) for w in (Wq, Wk, Wv, Wo))
    bq, bk, bv, bo = (np.asarray(v, dtype=np.float32) for v in (bq, bk, bv, bo))

    xT = np.ascontiguousarray(x.reshape(TOK, D).T)
    # Wo rows permuted: gathered A'^T row r*64+t of head-block h corresponds
    # to head (2r+h), dim t -> original Wo row r*128 + h*64 + t.
    wo4 = Wo.reshape(N_CORES, 2, HD, D)
    wo_perm = np.concatenate([wo4[:, 0], wo4[:, 1]], axis=0).reshape(D, D)

    in_maps = []
    for c in range(N_CORES):
        cs = slice(c * CW, (c + 1) * CW)
        in_maps.append({
            "xT": xT,
            "wq": np.ascontiguousarray(Wq[:, cs]),
            "wk": np.ascontiguousarray(Wk[:, cs]),
            "wv": np.ascontiguousarray(Wv[:, cs]),
            "wo": np.ascontiguousarray(wo_perm[:, cs]),
            "bq": np.ascontiguousarray(bq[cs].reshape(CW, 1)),
            "bk": np.ascontiguousarray(bk[cs].reshape(CW, 1)),
            "bv": np.ascontiguousarray(bv[cs].reshape(CW, 1)),
            "bo": np.ascontiguousarray(bo[cs].reshape(CW, 1)),
            "ident": np.eye(128, dtype=np.float32),
        })
    return in_maps


def kernel(x, Wq, bq, Wk, bk, Wv, bv, Wo, bo):
    from concourse import bass_utils

    in_maps = _make_in_maps(x, Wq, bq, Wk, bk, Wv, bv, Wo, bo)
    nc = _get_nc()
    res = bass_utils.run_bass_kernel_spmd(nc, in_maps,
                                          core_ids=list(range(N_CORES)))
    _CACHE["last_results"] = res

    out = np.empty((TOK, D), dtype=np.float32)
    for c in range(N_CORES):
        out[:, c * CW:(c + 1) * CW] = res.results[c]["outT"].T
    return out.reshape(B, S, D)


# revision 27
# speedup vs baseline: 1.1161x; 1.1161x over previous
"""Multi-head self-attention (B=4, S=2048, D=1024, H=16, Hd=64) on 8 TRN2 cores.

Sharding: tensor-parallel over heads. Core c owns heads 2c, 2c+1:
  - computes Q^T/K^T [128hd, tok] (f32r) and V (bf16) for its 2 heads
  - flash-style attention per (batch, head): S^T = K^T.T @ Q^T into
    [128,1024] psum stripes, exp on ACT (scale=1/8 folded in), AV +
    denominator via ones-append (M=65), reciprocal_approx_fast +
    PE ones-broadcast, normalize on DVE -> A^T (bf16)
  - 8 pipelined AllGathers (one per batch x local-head row-block)
  - out-proj: out^T[:, c-slice] = wo_perm_c.T @ A'^T + bo_c (bf16 matmul,
    fp32 psum); host transposes/concats column slices.

Matmul dtype: float32r (~1.5e-4 rel err, full PE rate at N>=512) for
projections/scores; bf16 for the P/V/AV/out-proj path (psum always fp32).
"""
import numpy as np

B, S, D, H, HD = 4, 2048, 1024, 16, 64
N_CORES = 8
TOK = B * S            # 8192
HPC = H // N_CORES     # 2 heads per core
CW = HPC * HD          # 128 cols per core
QS = 1024              # query stripe
NKT = S // 128         # 16 kt chunks per batch
NQS = S // QS          # 2 q stripes per batch
NTB = TOK // 512       # 16 token blocks overall

_CACHE = {}


def _build():
    import concourse.bacc as bacc
    import concourse.mybir as mybir
    import concourse.tile as tile

    F32 = mybir.dt.float32
    F32R = mybir.dt.float32r
    BF16 = mybir.dt.bfloat16
    AF = mybir.ActivationFunctionType

    nc = bacc.Bacc(trn_type="TRN2", target_bir_lowering=False, debug=False,
                   num_devices=N_CORES)

    xT = nc.dram_tensor("xT", [D, TOK], F32, kind="ExternalInput")
    wq = nc.dram_tensor("wq", [D, CW], F32, kind="ExternalInput")
    wk = nc.dram_tensor("wk", [D, CW], F32, kind="ExternalInput")
    wv = nc.dram_tensor("wv", [D, CW], F32, kind="ExternalInput")
    wo = nc.dram_tensor("wo", [D, CW], F32, kind="ExternalInput")  # row-permuted
    bq = nc.dram_tensor("bq", [CW, 1], F32, kind="ExternalInput")
    bk = nc.dram_tensor("bk", [CW, 1], F32, kind="ExternalInput")
    bv = nc.dram_tensor("bv", [CW, 1], F32, kind="ExternalInput")
    bo = nc.dram_tensor("bo", [CW, 1], F32, kind="ExternalInput")
    ident = nc.dram_tensor("ident", [128, 128], F32, kind="ExternalInput")
    outT = nc.dram_tensor("outT", [CW, TOK], F32, kind="ExternalOutput")

    with tile.TileContext(nc) as tc:
        with tc.tile_pool(name="sb", bufs=1) as sb, \
             tc.tile_pool(name="dram", bufs=1, space="DRAM") as dram:
            # ---------------- prologue: weights, biases, constants --------
            w_r = {}
            for wname, wdram, odt in (("wq", wq, F32R), ("wk", wk, F32R),
                                      ("wv", wv, F32R), ("wo", wo, BF16)):
                wr = sb.tile([128, D], odt, tag=f"{wname}_r",
                             name=f"{wname}_r")
                wsrc = wdram.ap().rearrange("(k p) m -> p k m", p=128)
                for half in range(2):
                    wst = sb.tile([128, 512], F32, tag="xstage", bufs=6,
                                  name=f"wst_{wname}{half}")
                    nc.sync.dma_start(
                        wst[:].rearrange("p (k m) -> p k m", k=4),
                        wsrc[:, half * 4:half * 4 + 4, :])
                    nc.vector.tensor_copy(
                        wr[:, half * 512:(half + 1) * 512], wst[:])
                w_r[wname] = wr
            wq_r, wk_r, wv_r, wo_b = w_r["wq"], w_r["wk"], w_r["wv"], w_r["wo"]

            idst = sb.tile([128, 512], F32, tag="xstage", bufs=6, name="idst")
            nc.sync.dma_start(idst[:, 0:128], ident[:])
            identr = sb.tile([128, 128], F32R, tag="identr", name="identr")
            nc.vector.tensor_copy(identr[:], idst[:, 0:128])

            bias_t = {}
            for bname, bdram in (("bq", bq), ("bk", bk), ("bv", bv),
                                 ("bo", bo)):
                bt_ = sb.tile([CW, 1], F32, tag=f"{bname}_t", name=f"{bname}_t")
                nc.sync.dma_start(bt_[:], bdram[:])
                bias_t[bname] = bt_

            ones_f = sb.tile([65, 64], F32, tag="ones_f", name="ones_f")
            nc.vector.memset(ones_f[:], 1.0)
            ones_r = sb.tile([65, 64], F32R, tag="ones_r", name="ones_r")
            nc.vector.tensor_copy(ones_r[:], ones_f[:])

            agin = {}
            agout = {}
            for b in range(B):
                for h in range(2):
                    for q in range(NQS):
                        agin[(b, h, q)] = dram.tile(
                            [64, QS], BF16, tag=f"agi{b}{h}{q}",
                            name=f"agi{b}{h}{q}")
                        agout[(b, h, q)] = dram.tile(
                            [64 * N_CORES, QS], BF16, tag=f"ago{b}{h}{q}",
                            addr_space="Shared", name=f"ago{b}{h}{q}")

            with tc.tile_pool(name="ps12", bufs=1, space="PSUM") as ps:
                qkv = {}
                xr_tiles = {}
                vext = {}
                at_tiles = {}

                def emit_p1_loads(b, tb):
                    if tb == 0:
                        qkv[b] = (
                            sb.tile([128, S], F32R, tag="qt_sb", bufs=2,
                                    name=f"qt{b}"),
                            sb.tile([128, S], F32R, tag="kt_sb", bufs=2,
                                    name=f"kt{b}"),
                            sb.tile([128, S], F32R, tag="vt_sb", bufs=2,
                                    name=f"vt{b}"),
                        )
                    g0 = b * S + tb * 512
                    xr = []
                    for k in range(8):
                        xs = sb.tile([128, 512], F32, tag="xstage", bufs=6,
                                     name=f"xs{b}_{tb}_{k}")
                        nc.sync.dma_start(
                            xs[:], xT[k * 128:(k + 1) * 128, g0:g0 + 512])
                        xk = sb.tile([128, 512], F32R, tag="xr", bufs=18,
                                     name=f"xr{b}_{tb}_{k}")
                        nc.vector.tensor_copy(xk[:], xs[:])
                        xr.append(xk)
                    xr_tiles[(b, tb)] = xr

                def emit_p1_group(b, tb, which):
                    qt, kt, vt = qkv[b]
                    xr = xr_tiles[(b, tb)]
                    w_, out_sb, bias = (
                        (wq_r, qt, bias_t["bq"]),
                        (wk_r, kt, bias_t["bk"]),
                        (wv_r, vt, bias_t["bv"]))[which]
                    pp = ps.tile([128, 512], F32, tag="proj", bufs=2,
                                 name=f"pp{b}_{tb}_{which}")
                    for k in range(8):
                        nc.tensor.matmul(
                            pp[:], w_[:, k * 128:(k + 1) * 128],
                            xr[k][:], start=(k == 0), stop=(k == 7))
                    nc.vector.tensor_scalar_add(
                        out_sb[:, tb * 512:(tb + 1) * 512], pp[:], bias[:])

                def emit_vext_chunk(b, tbi):
                    vt = qkv[b][2]
                    for ktc in range(4 * tbi, 4 * tbi + 4):
                        tp = ps.tile([128, 128], F32R, tag="proj", bufs=2,
                                     name=f"tp{b}_{ktc}")
                        nc.tensor.transpose(
                            tp[:], vt[:, ktc * 128:(ktc + 1) * 128],
                            identr[:])
                        for h in range(2):
                            ve = sb.tile([128, 128], BF16, tag="vext",
                                         bufs=36, name=f"ve{b}_{ktc}_{h}")
                            nc.vector.memset(ve[:, 0:64], 1.0)
                            nc.vector.tensor_copy(
                                ve[:, 64:128],
                                tp[:, h * 64:(h + 1) * 64])
                            vext[(b, ktc, h)] = ve

                def emit_p2_stripe(b, h, qs_i, jobs):
                    qt, kt, vt = qkv[b]
                    if qs_i == 0:
                        at_tiles[(b, h)] = sb.tile(
                            [128, S], BF16, tag="at_t", bufs=2,
                            name=f"at{b}_{h}")
                    at_t = at_tiles[(b, h)]
                    q0 = qs_i * QS
                    pav = ps.tile([128, QS], F32, tag="av", bufs=1,
                                  name=f"pav{b}_{h}_{qs_i}")
                    def emit_av(ktc, pt):
                        for half in range(2):
                            nc.tensor.matmul(
                                pav[:, half * 512:(half + 1) * 512],
                                vext[(b, ktc, h)][:],
                                pt[:, half * 512:(half + 1) * 512],
                                start=(ktc == 0), stop=(ktc == NKT - 1))

                    for ktc in range(NKT):
                        if ktc in jobs:
                            jobs[ktc]()
                        s_ps = ps.tile([128, QS], F32, tag="s", bufs=2,
                                       name=f"s{b}{h}{qs_i}{ktc}")
                        for half in range(2):
                            nc.tensor.matmul(
                                s_ps[:, half * 512:(half + 1) * 512],
                                kt[h * 64:(h + 1) * 64,
                                   ktc * 128:(ktc + 1) * 128],
                                qt[h * 64:(h + 1) * 64,
                                   q0 + half * 512:q0 + (half + 1) * 512],
                                start=True, stop=True,
                                tile_position=(h * 64, 0))
                        pt = sb.tile([128, QS], BF16, tag="p_sb",
                                     bufs=3, name=f"p{b}{h}{qs_i}{ktc}")
                        nc.scalar.activation(pt[:], s_ps[:], AF.Exp,
                                             scale=0.125)
                        emit_av(ktc, pt)
                    araw = sb.tile([128, QS], F32, tag="araw", bufs=3,
                                   name=f"ar{b}_{h}_{qs_i}")
                    nc.vector.tensor_copy(araw[:], pav[:])
                    rcf = sb.tile([128, QS], F32, tag="rcf", bufs=3,
                                  name=f"rcf{b}_{h}_{qs_i}")
                    nc.vector.reciprocal_approx_fast(rcf[:], araw[:])
                    bcs = sb.tile([128, QS], F32, tag="bcs", bufs=2,
                                  name=f"bcs{b}_{h}_{qs_i}")
                    nc.gpsimd.partition_broadcast(bcs[:], rcf[0:1, :])
                    nc.vector.tensor_mul(at_t[64:128, q0:q0 + QS],
                                         araw[64:128, :], bcs[64:128, :])
                    nc.sync.dma_start(agin[(b, h, qs_i)][:],
                                      at_t[64:128, q0:q0 + QS])
                    nc.gpsimd.collective_compute(
                        "AllGather", mybir.AluOpType.bypass,
                        replica_groups=[list(range(N_CORES))],
                        ins=[agin[(b, h, qs_i)][:]],
                        outs=[agout[(b, h, qs_i)][:]],
                    )

                def emit_p3_tb(tb):
                    bb = tb // 4
                    qsb = (tb % 4) // 2
                    hf = tb % 2
                    c0 = hf * 512
                    po = ps.tile([128, 512], F32, tag="proj", bufs=2,
                                 name=f"po{tb}")
                    for kc in range(8):
                        ast = sb.tile([128, 512], BF16, tag="ast", bufs=6,
                                      name=f"ast{tb}_{kc}")
                        src = agout[(bb, kc // 4, qsb)]
                        r0 = (kc % 4) * 128
                        nc.sync.dma_start(ast[:],
                                          src[r0:r0 + 128, c0:c0 + 512])
                        nc.tensor.matmul(po[:],
                                         wo_b[:, kc * 128:(kc + 1) * 128],
                                         ast[:], start=(kc == 0),
                                         stop=(kc == 7))
                    ot = sb.tile([128, 512], F32, tag="ot", bufs=3,
                                 name=f"ot{tb}")
                    nc.vector.tensor_scalar_add(ot[:], po[:], bias_t["bo"][:])
                    nc.sync.dma_start(outT[:, tb * 512:(tb + 1) * 512], ot[:])

                # batch 0 projections up front
                for tb in range(4):
                    emit_p1_loads(0, tb)
                    for w in range(3):
                        emit_p1_group(0, tb, w)
                for tbi in range(4):
                    emit_vext_chunk(0, tbi)

                for b in range(B):
                    for h in range(2):
                        for qs_i in range(NQS):
                            i = h * NQS + qs_i
                            jobs = {}
                            # P3 token-blocks woven into later batches' stripes
                            p3_sched = {(1, 1): [0], (1, 2): [1], (1, 3): [2],
                                        (2, 0): [3], (2, 1): [4], (2, 2): [5],
                                        (2, 3): [6], (3, 0): [7],
                                        (3, 1): [8, 9], (3, 2): [10, 11],
                                        (3, 3): [12, 13]}
                            for j, tb3 in enumerate(p3_sched.get((b, i), [])):
                                jobs[5 + 8 * j] = (lambda tb=tb3:
                                                   emit_p3_tb(tb))
                            if b + 1 < B:
                                jobs[0] = (lambda bb=b + 1, tb=i:
                                           emit_p1_loads(bb, tb))
                                if i >= 1:
                                    jobs[3] = (lambda bb=b + 1, tb=i - 1:
                                               emit_p1_group(bb, tb, 0))
                                    jobs[7] = (lambda bb=b + 1, tb=i - 1:
                                               emit_p1_group(bb, tb, 1))
                                    jobs[11] = (lambda bb=b + 1, tb=i - 1:
                                                emit_p1_group(bb, tb, 2))
                                if i >= 2:
                                    jobs[14] = (lambda bb=b + 1, tbi=i - 2:
                                                emit_vext_chunk(bb, tbi))
                            emit_p2_stripe(b, h, qs_i, jobs)
                    if b + 1 < B:
                        # tail of next batch's projections
                        for w in range(3):
                            emit_p1_group(b + 1, 3, w)
                        emit_vext_chunk(b + 1, 2)
                        emit_vext_chunk(b + 1, 3)

                # ------------- P3 tail: last token blocks -----------------
                for tb in range(14, NTB):
                    emit_p3_tb(tb)

    nc.compile()
    return nc


def _get_nc():
    if "nc" not in _CACHE:
        _CACHE["nc"] = _build()
    return _CACHE["nc"]


def _make_in_maps(x, Wq, bq, Wk, bk, Wv, bv, Wo, bo):
    x = np.asarray(x, dtype=np.float32)
    Wq, Wk, Wv, Wo = (np.asarray(w, dtype=np.float32) for w in (Wq, Wk, Wv, Wo))
    bq, bk, bv, bo = (np.asarray(v, dtype=np.float32) for v in (bq, bk, bv, bo))

    xT = np.ascontiguousarray(x.reshape(TOK, D).T)
    # Wo rows permuted: gathered A'^T row r*64+t of head-block h corresponds
    # to head (2r+h), dim t -> original Wo row r*128 + h*64 + t.
    wo4 = Wo.reshape(N_CORES, 2, HD, D)
    wo_perm = np.concatenate([wo4[:, 0], wo4[:, 1]], axis=0).reshape(D, D)

    in_maps = []
    for c in range(N_CORES):
        cs = slice(c * CW, (c + 1) * CW)
        in_maps.append({
            "xT": xT,
            "wq": np.ascontiguousarray(Wq[:, cs]),
            "wk": np.ascontiguousarray(Wk[:, cs]),
            "wv": np.ascontiguousarray(Wv[:, cs]),
            "wo": np.ascontiguousarray(wo_perm[:, cs]),
            "bq": np.ascontiguousarray(bq[cs].reshape(CW, 1)),
            "bk": np.ascontiguousarray(bk[cs].reshape(CW, 1)),
            "bv": np.ascontiguousarray(bv[cs].reshape(CW, 1)),
            "bo": np.ascontiguousarray(bo[cs].reshape(CW, 1)),
            "ident": np.eye(128, dtype=np.float32),
        })
    return in_maps


def kernel(x, Wq, bq, Wk, bk, Wv, bv, Wo, bo):
    from concourse import bass_utils

    in_maps = _make_in_maps(x, Wq, bq, Wk, bk, Wv, bv, Wo, bo)
    nc = _get_nc()
    res = bass_utils.run_bass_kernel_spmd(nc, in_maps,
                                          core_ids=list(range(N_CORES)))
    _CACHE["last_results"] = res

    out = np.empty((TOK, D), dtype=np.float32)
    for c in range(N_CORES):
        out[:, c * CW:(c + 1) * CW] = res.results[c]["outT"].T
    return out.reshape(B, S, D)
